# revision 1
# baseline (speedup 1.0000x reference)

# Trainium2 Bass kernel for nn_CameraAwareLoss (self-contained).
#
# Strategy (8 NeuronCores, data-parallel over groups):
#   - 16384 rows = 4096 groups x 4 samples, d=1024. Each core owns 512 groups.
#   - Loop1: normalize rows (fa, bf16), group centers via norm-weighted
#     indicator matmul on PE, l2-normalize centers (fc, bf16); ship fa+fc.
#   - AllGather fc (1MB/rank) and fa (4MB/rank) immediately; local slot
#     stats (loop2) overlap the collectives.
#   - dist matmul in bf16 (512x4096x1024 per core) + same-label mask fused
#     into the PSUM drain; argmin via max8/find_index8.
#   - indirect-DMA gather of hard-negative groups; per-group 4x4/8x8 stats
#     via the camera-slot identity on PE and 16-pair products on DVE.
#   - Per-core partial loss summed on host.
import numpy as np
import ml_dtypes

import concourse.bass as bass
import concourse.mybir as mybir
import concourse.bacc as bacc
from concourse import tile
from concourse.bass_utils import run_bass_kernel_spmd

NCORES = 8
NG = 4096          # total groups
G = NG // NCORES   # groups per core (512)
D = 1024
R = G * 4          # rows per core (2048)
RT = R // 128      # row tiles per core (16)
PT = G // 128      # group tiles per core (4)
KT = D // 128      # contraction tiles (8)
BIG = 1e6
MARGIN = 0.3

f32 = mybir.dt.float32
bf16 = mybir.dt.bfloat16
u16 = mybir.dt.uint16
u32 = mybir.dt.uint32

_CACHE = {}

AF = mybir.ActivationFunctionType
OP = mybir.AluOpType


def _build(stage=4):
    nc = bacc.Bacc("TRN2", target_bir_lowering=False, debug=False,
                   num_devices=NCORES)

    x_sh = nc.dram_tensor("x_sh", [R, D], f32, kind="ExternalInput")
    lab_bc = nc.dram_tensor("lab_bc", [128, NG], u16, kind="ExternalInput")
    lab_loc = nc.dram_tensor("lab_loc", [G, 1], f32, kind="ExternalInput")
    gtab = nc.dram_tensor("gtab", [NG, 16], f32, kind="ExternalInput")
    atab = nc.dram_tensor("atab", [G, 16], f32, kind="ExternalInput")
    e4b_in = nc.dram_tensor("e4b", [4, 128, 128], bf16, kind="ExternalInput")
    e4f_in = nc.dram_tensor("e4f", [4, 128, 128], f32, kind="ExternalInput")
    wslot_in = nc.dram_tensor("wslot", [RT, 128, 128], bf16, kind="ExternalInput")
    ones_in = nc.dram_tensor("ones1", [128, 1], f32, kind="ExternalInput")
    ident_in = nc.dram_tensor("ident", [128, 128], bf16, kind="ExternalInput")

    loss_out = nc.dram_tensor("loss_part", [1, 1], f32, kind="ExternalOutput")

    PR = R + G  # payload rows per rank (2560)
    pay_full = nc.dram_tensor("pay_full", [NCORES * PR, D], bf16, kind="Internal",
                              addr_space="Shared")
    ma_full = nc.dram_tensor("ma_full", [NG, 1], f32, kind="Internal",
                             addr_space="Shared")

    rg = [list(range(NCORES))]

    from contextlib import ExitStack
    with tile.TileContext(nc) as tc:
        with ExitStack() as stack:
            ep = stack.enter_context
            ct = ep(tc.tile_pool(name="consts", bufs=1))
            px = ep(tc.tile_pool(name="px", bufs=3))
            psq = ep(tc.tile_pool(name="psq", bufs=3))
            pfa = ep(tc.tile_pool(name="pfa", bufs=RT))
            pew = ep(tc.tile_pool(name="pew", bufs=3))
            pfc = ep(tc.tile_pool(name="pfc", bufs=2))
            plhs = ep(tc.tile_pool(name="plhs", bufs=KT))
            prhs = ep(tc.tile_pool(name="prhs", bufs=KT))
            psim = ep(tc.tile_pool(name="psim", bufs=2))
            pga = ep(tc.tile_pool(name="pga", bufs=1))
            pgb = ep(tc.tile_pool(name="pgb", bufs=1))
            pprod = ep(tc.tile_pool(name="pprod", bufs=1))
            psmall = ep(tc.tile_pool(name="psmall", bufs=4))
            pnrm = ep(tc.tile_pool(name="pnrm", bufs=8))
            pslotsq = ep(tc.tile_pool(name="pslotsq", bufs=RT))
            pma = ep(tc.tile_pool(name="pma", bufs=PT))
            ppc = ep(tc.tile_pool(name="ppc", bufs=2, space="PSUM"))
            ppm = ppc
            ptiny = ep(tc.tile_pool(name="ptiny", bufs=2, space="PSUM"))
            pdram = ep(tc.tile_pool(name="pdram", bufs=1, space="DRAM"))
            pay = pdram.tile([PR, D], bf16, tag="pay")
            ma_bounce = pdram.tile([G, 1], f32, tag="mab")

            # ---- constants to SBUF ----
            e4b = ct.tile([128, 4 * 128], bf16, tag="e4b")
            e4f = ct.tile([128, 4 * 128], f32, tag="e4f")
            for j in range(4):
                nc.sync.dma_start(e4b[:, 128 * j:128 * (j + 1)], e4b_in[j, :, :])
                nc.sync.dma_start(e4f[:, 128 * j:128 * (j + 1)], e4f_in[j, :, :])
            wsl = ct.tile([128, RT * 128], bf16, tag="wsl")
            for t in range(RT):
                nc.sync.dma_start(wsl[:, 128 * t:128 * (t + 1)], wslot_in[t, :, :])
            ones_sb = ct.tile([128, 1], f32, tag="ones")
            nc.sync.dma_start(ones_sb[:], ones_in[:])
            ident_sb = ct.tile([128, 128], bf16, tag="ident")
            nc.sync.dma_start(ident_sb[:], ident_in[:])
            fcT = []
            for kk in range(KT):
                ft = plhs.tile([128, G], bf16, tag="lhs", name=f"fcT{kk}")
                fcT.append(ft)
            labbc_sb = ct.tile([128, NG], u16, tag="labbc")
            nc.sync.dma_start(labbc_sb[:], lab_bc[:])
            lab_sb = ct.tile([128, PT], f32, tag="labloc")
            atab_sb = ct.tile([128, 16 * PT], f32, tag="atab")
            for p in range(PT):
                nc.sync.dma_start(lab_sb[:, p:p + 1], lab_loc[128 * p:128 * (p + 1), :])
                nc.sync.dma_start(atab_sb[:, 16 * p:16 * (p + 1)],
                                  atab[128 * p:128 * (p + 1), :])

            fa_tiles = []   # RT tiles
            ma_tiles = []

            # ---- loop1: normalize + centers; ship fa and fc ASAP ----
            for p in range(PT):
                pc = ppc.tile([128, D], f32, tag="big")
                for q in range(4):
                    rt = 4 * p + q
                    x_t = px.tile([128, D], f32, tag="x")
                    nc.sync.dma_start(x_t[:], x_sh[128 * rt:128 * (rt + 1), :])
                    sq_t = psq.tile([128, D], bf16, tag="sq")
                    ssq = pnrm.tile([128, 1], f32, tag="ssq")
                    nc.scalar.activation(sq_t[:], x_t[:], AF.Square,
                                         accum_out=ssq[:])
                    nm = pnrm.tile([128, 1], f32, tag="nm")
                    nc.scalar.activation(nm[:], ssq[:], AF.Sqrt)
                    rn = pnrm.tile([128, 1], f32, tag="rn")
                    nc.vector.reciprocal(rn[:], nm[:])
                    fa_t = pfa.tile([128, D], bf16, tag="fa")
                    nc.scalar.activation(fa_t[:], x_t[:], AF.Copy, scale=rn[:])
                    ew_t = pew.tile([128, 128], bf16, tag="ew")
                    nc.vector.tensor_scalar(ew_t[:], e4b[:, 128 * q:128 * (q + 1)],
                                            nm[:], None, OP.mult)
                    nc.sync.dma_start(pay[128 * rt:128 * (rt + 1), :], fa_t[:])
                    for h in range(2):
                        nc.tensor.matmul(pc[:, 512 * h:512 * (h + 1)],
                                         lhsT=ew_t[:],
                                         rhs=fa_t[:, 512 * h:512 * (h + 1)],
                                         start=(q == 0), stop=(q == 3))
                    fa_tiles.append(fa_t)
                csq = psq.tile([128, D], bf16, tag="sq")
                cn2 = pnrm.tile([128, 1], f32, tag="cn2")
                nc.scalar.activation(csq[:], pc[:], AF.Square, accum_out=cn2[:])
                cnm = pnrm.tile([128, 1], f32, tag="cnm")
                nc.scalar.activation(cnm[:], cn2[:], AF.Sqrt)
                crn = pnrm.tile([128, 1], f32, tag="crn")
                nc.vector.reciprocal(crn[:], cnm[:])
                fc_t = pfc.tile([128, D], bf16, tag="fc")
                nc.vector.tensor_scalar(fc_t[:], pc[:], crn[:], None, OP.mult)
                for kk in range(KT):
                    tp_ps = ptiny.tile([128, 128], bf16, tag="tp")
                    nc.tensor.transpose(tp_ps[:], fc_t[:, 128 * kk:128 * (kk + 1)],
                                        ident_sb[:])
                    nc.vector.tensor_copy(fcT[kk][:, 128 * p:128 * (p + 1)], tp_ps[:])

            # ---- ship fcT region; single merged AllGather ----
            for kk in range(KT):
                nc.sync.dma_start(
                    pay[R + 64 * kk:R + 64 * (kk + 1), :].rearrange(
                        "a (x c) -> (a x) c", x=2),
                    fcT[kk][:])
            if stage >= 2:
                nc.gpsimd.collective_compute("AllGather", OP.bypass,
                                             replica_groups=rg,
                                             ins=[pay[:]], outs=[pay_full[:]])

            # ---- loop2: slot stats (overlaps the collectives) ----
            for p in range(PT):
                slotsq_p = []
                for q in range(4):
                    rt = 4 * p + q
                    ps = ppc.tile([128, D], f32, tag="big")
                    for h in range(2):
                        nc.tensor.matmul(ps[:, 512 * h:512 * (h + 1)],
                                         lhsT=wsl[:, 128 * rt:128 * (rt + 1)],
                                         rhs=fa_tiles[rt][:, 512 * h:512 * (h + 1)],
                                         start=True, stop=True)
                    ssq_t = psq.tile([128, D], bf16, tag="sq")
                    slq = pslotsq.tile([128, 1], f32, tag="slotsq")
                    nc.scalar.activation(ssq_t[:], ps[:], AF.Square,
                                         accum_out=slq[:])
                    slotsq_p.append(slq)
                pT = ppc.tile([128, D], f32, tag="big")
                for q in range(4):
                    for h in range(2):
                        nc.tensor.matmul(pT[:, 512 * h:512 * (h + 1)],
                                         lhsT=e4b[:, 128 * q:128 * (q + 1)],
                                         rhs=fa_tiles[4 * p + q][:, 512 * h:512 * (h + 1)],
                                         start=(q == 0), stop=(q == 3))
                tsq_t = psq.tile([128, D], bf16, tag="sq")
                tsq = pnrm.tile([128, 1], f32, tag="tsq")
                nc.scalar.activation(tsq_t[:], pT[:], AF.Square, accum_out=tsq[:])
                pts = ptiny.tile([128, 1], f32, tag="tiny")
                for q in range(4):
                    nc.tensor.matmul(pts[:], lhsT=e4f[:, 128 * q:128 * (q + 1)],
                                     rhs=slotsq_p[q][:],
                                     start=(q == 0), stop=(q == 3))
                s1m = pnrm.tile([128, 1], f32, tag="s1m")
                nc.vector.tensor_tensor(s1m[:], pts[:], tsq[:], OP.subtract)
                ma_t = pma.tile([128, 1], f32, tag="ma")
                nc.vector.tensor_tensor(ma_t[:], s1m[:],
                                        atab_sb[:, 16 * p + 11:16 * p + 12],
                                        OP.mult)
                ma_tiles.append(ma_t)
                nc.sync.dma_start(ma_bounce[128 * p:128 * (p + 1), :], ma_t[:])

            if stage == 1:
                pts1 = ptiny.tile([1, 1], f32, tag="tiny")
                for p in range(PT):
                    nc.tensor.matmul(pts1[:], lhsT=ones_sb[:], rhs=ma_tiles[p][:],
                                     start=(p == 0), stop=(p == PT - 1))
                lsb1 = psmall.tile([1, 1], f32, tag="lsb")
                nc.scalar.copy(lsb1[:], pts1[:])
                nc.sync.dma_start(loss_out[:], lsb1[:])

            if stage == 2:
                mchk = psmall.tile([128, 1], f32, tag="machk")
                nc.sync.dma_start(mchk[:], ma_full[0:128, :])
                pts2 = ptiny.tile([1, 1], f32, tag="tiny")
                nc.tensor.matmul(pts2[:], lhsT=ones_sb[:], rhs=mchk[:],
                                 start=True, stop=True)
                lsb2 = psmall.tile([1, 1], f32, tag="lsb")
                nc.scalar.copy(lsb2[:], pts2[:])
                nc.sync.dma_start(loss_out[:], lsb2[:])

            # ---- center loads (plain DMA from gathered fcT regions) ----
            if stage >= 3:
                lhsT = fcT
                rhsT = []
                for kk in range(KT):
                    rt_ = prhs.tile([128, NG], bf16, tag="rhs")
                    for r in range(NCORES):
                        base = PR * r + R + 64 * kk
                        nc.sync.dma_start(
                            rt_[:, G * r:G * (r + 1)],
                            pay_full[base:base + 64, :].rearrange(
                                "a (x c) -> (a x) c", x=2))
                    rhsT.append(rt_)
            if stage >= 2:
                nc.gpsimd.collective_compute("AllGather", OP.bypass,
                                             replica_groups=rg,
                                             ins=[ma_bounce[:]], outs=[ma_full[:]])


            # ---- dist matmul + argmin + stats per group-tile ----
            ploss = ptiny.tile([1, 1], f32, tag="tiny", name="ploss") if stage >= 3 else None
            for mt in range(PT if stage >= 3 else 0):
                sim = psim.tile([128, NG], f32, tag="sim")
                nc.vector.tensor_scalar(sim[:], labbc_sb[:],
                                        lab_sb[:, mt:mt + 1], -BIG,
                                        OP.is_equal, OP.mult)
                for ch in range(4):
                    pm_t = ppm.tile([128, 1024], f32, tag="big")
                    for h in range(2):
                        for kk in range(KT):
                            nc.tensor.matmul(
                                pm_t[:, 512 * h:512 * (h + 1)],
                                lhsT=lhsT[kk][:, 128 * mt:128 * (mt + 1)],
                                rhs=rhsT[kk][:, 1024 * ch + 512 * h:
                                             1024 * ch + 512 * (h + 1)],
                                start=(kk == 0), stop=(kk == KT - 1))
                    nc.vector.tensor_tensor(sim[:, 1024 * ch:1024 * (ch + 1)],
                                            pm_t[:],
                                            sim[:, 1024 * ch:1024 * (ch + 1)],
                                            OP.add)
                mx8 = psmall.tile([128, 8], f32, tag="mx8")
                mi8 = psmall.tile([128, 8], u32, tag="mi8")
                nc.vector.max_with_indices(mx8[:], mi8[:], sim[:])
                idx = mi8[:, 0:1]

                if stage == 3:
                    nc.tensor.matmul(ploss[:], lhsT=ones_sb[:], rhs=mi8[:, 0:1],
                                     start=(mt == 0), stop=(mt == PT - 1))
                    if mt == PT - 1:
                        lsb3 = psmall.tile([1, 1], f32, tag="lsb")
                        nc.scalar.copy(lsb3[:], ploss[:])
                        nc.sync.dma_start(loss_out[:], lsb3[:])
                    continue

                idxr = psmall.tile([128, 1], u32, tag="idxr")
                nc.vector.tensor_scalar(idxr[:], idx, 9, None,
                                        OP.arith_shift_right)
                nc.vector.tensor_scalar(idxr[:], idxr[:], 128, None, OP.mult)
                nc.vector.tensor_tensor(idxr[:], idxr[:], idx, OP.add)
                gB = pgb.tile([128, 4 * D], bf16, tag="gb")
                nc.gpsimd.indirect_dma_start(
                    out=gB[:], out_offset=None,
                    in_=pay_full[:].rearrange("(a x) c -> a (x c)", x=4),
                    in_offset=bass.IndirectOffsetOnAxis(ap=idxr[:], axis=0))
                btab = psmall.tile([128, 16], f32, tag="btab")
                nc.gpsimd.indirect_dma_start(
                    out=btab[:], out_offset=None, in_=gtab[:],
                    in_offset=bass.IndirectOffsetOnAxis(ap=idx, axis=0))
                bma = psmall.tile([128, 1], f32, tag="bma")
                nc.gpsimd.indirect_dma_start(
                    out=bma[:], out_offset=None, in_=ma_full[:],
                    in_offset=bass.IndirectOffsetOnAxis(ap=idx, axis=0))
                gA = pga.tile([128, 4 * D], bf16, tag="ga")
                nc.sync.dma_start(
                    gA[:],
                    pay[512 * mt:512 * (mt + 1), :].rearrange(
                        "(g r) d -> g (r d)", r=4))

                # cross products PAB[p, 4i+j] = A_i . B_j
                pab = psmall.tile([128, 16], f32, tag="pab")
                for i in range(4):
                    pr = pprod.tile([128, 4 * D], bf16, tag="prod")
                    pr3 = pr[:].rearrange("p (j d) -> p j d", j=4)
                    a3 = gA[:, D * i:D * (i + 1)].rearrange("p (o d) -> p o d", o=1)
                    b3 = gB[:].rearrange("p (j d) -> p j d", j=4)
                    a3b, b3b = bass.broadcast_tensor_aps(a3, b3)
                    nc.vector.tensor_tensor(pr3, a3b, b3b, OP.mult)
                    nc.vector.tensor_reduce(
                        pab[:, 4 * i:4 * (i + 1)].rearrange("p (j o) -> p j o", o=1),
                        pr3, mybir.AxisListType.X, OP.add)
                eq16 = psmall.tile([128, 16], f32, tag="eq16")
                acam = atab_sb[:, 16 * mt:16 * mt + 4].rearrange("p (c o) -> p c o", o=1)
                bcam = btab[:, 0:4].rearrange("p (o c) -> p o c", o=1)
                acb, bcb = bass.broadcast_tensor_aps(acam, bcam)
                nc.vector.tensor_tensor(eq16[:].rearrange("p (i j) -> p i j", j=4),
                                        acb, bcb, OP.is_equal)
                scr16 = psmall.tile([128, 16], f32, tag="scr16")
                sumeq = psmall.tile([128, 1], f32, tag="sumeq")
                nc.vector.tensor_tensor(scr16[:], pab[:], eq16[:], OP.mult)
                nc.vector.tensor_reduce(sumeq[:], scr16[:], mybir.AxisListType.X,
                                        OP.add)
                sumall = psmall.tile([128, 1], f32, tag="sumall")
                nc.vector.tensor_reduce(sumall[:], pab[:], mybir.AxisListType.X,
                                        OP.add)
                scr6 = psmall.tile([128, 6], f32, tag="scr6")
                cntdot = psmall.tile([128, 1], f32, tag="cntdot")
                nc.vector.tensor_tensor(scr6[:], atab_sb[:, 16 * mt + 4:16 * mt + 10],
                                        btab[:, 4:10], OP.mult)
                nc.vector.tensor_reduce(cntdot[:], scr6[:], mybir.AxisListType.X,
                                        OP.add)
                c2 = psmall.tile([128, 1], f32, tag="c2")
                nc.vector.tensor_scalar(c2[:], cntdot[:], -2.0, 32.0,
                                        OP.mult, OP.add)
                c2m = psmall.tile([128, 1], f32, tag="c2m")
                nc.vector.tensor_scalar(c2m[:], c2[:], 1.0, None, OP.max)
                rec2 = psmall.tile([128, 1], f32, tag="rec2")
                nc.vector.reciprocal(rec2[:], c2m[:])
                valid2 = psmall.tile([128, 1], f32, tag="valid2")
                nc.vector.tensor_scalar(valid2[:], c2[:], 0.0, None, OP.is_gt)
                scross = psmall.tile([128, 1], f32, tag="scross")
                nc.vector.tensor_tensor(scross[:], sumall[:], sumeq[:], OP.subtract)
                s2 = psmall.tile([128, 1], f32, tag="s2")
                nc.vector.tensor_scalar(s2[:], scross[:], -2.0, None, OP.mult)
                m2 = psmall.tile([128, 1], f32, tag="m2")
                nc.vector.tensor_tensor(m2[:], s2[:], rec2[:], OP.mult)
                va = atab_sb[:, 16 * mt + 10:16 * mt + 11]
                vb = btab[:, 10:11]
                wa = psmall.tile([128, 1], f32, tag="wa")
                nc.vector.tensor_scalar(wa[:], vb, -0.5, 1.0, OP.mult, OP.add)
                nc.vector.tensor_tensor(wa[:], wa[:], va, OP.mult)
                wb = psmall.tile([128, 1], f32, tag="wb")
                nc.vector.tensor_scalar(wb[:], va, -0.5, 1.0, OP.mult, OP.add)
                nc.vector.tensor_tensor(wb[:], wb[:], vb, OP.mult)
                m1a = psmall.tile([128, 1], f32, tag="m1a")
                nc.vector.tensor_tensor(m1a[:], wa[:], ma_tiles[mt][:], OP.mult)
                m1b = psmall.tile([128, 1], f32, tag="m1b")
                nc.vector.tensor_tensor(m1b[:], wb[:], bma[:], OP.mult)
                m1 = psmall.tile([128, 1], f32, tag="m1")
                nc.vector.tensor_tensor(m1[:], m1a[:], m1b[:], OP.add)
                diff = psmall.tile([128, 1], f32, tag="diff")
                nc.vector.tensor_tensor(diff[:], m1[:], m2[:], OP.subtract)
                nc.vector.tensor_scalar(diff[:], diff[:], MARGIN, None, OP.add)
                lossv = psmall.tile([128, 1], f32, tag="lossv")
                nc.scalar.activation(lossv[:], diff[:], AF.Relu)
                vor = psmall.tile([128, 1], f32, tag="vor")
                nc.vector.tensor_tensor(vor[:], va, vb, OP.mult)
                vsum = psmall.tile([128, 1], f32, tag="vsum")
                nc.vector.tensor_tensor(vsum[:], va, vb, OP.add)
                nc.vector.tensor_tensor(vor[:], vsum[:], vor[:], OP.subtract)
                nc.vector.tensor_tensor(lossv[:], lossv[:], vor[:], OP.mult)
                nc.vector.tensor_tensor(lossv[:], lossv[:], valid2[:], OP.mult)
                nc.tensor.matmul(ploss[:], lhsT=ones_sb[:], rhs=lossv[:],
                                 start=(mt == 0), stop=(mt == PT - 1))

            if stage >= 4:
                lsb = psmall.tile([1, 1], f32, tag="lsb")
                nc.scalar.copy(lsb[:], ploss[:])
                nc.sync.dma_start(loss_out[:], lsb[:])

    nc.compile()
    return nc


def _host_prep(input, target, camera_id):
    x = np.ascontiguousarray(np.asarray(input, dtype=np.float32))
    tgt = np.asarray(target).reshape(NG, 4)
    cam = np.asarray(camera_id).reshape(NG, 4)
    labels = tgt[:, 0].astype(np.int64)

    cnt = np.zeros((NG, 6), np.float32)
    for c in range(6):
        cnt[:, c] = (cam == c).sum(axis=1)
    c1 = 16.0 - (cnt * cnt).sum(axis=1)
    rec1 = 1.0 / np.maximum(c1, 1.0)
    va = (c1 > 0).astype(np.float32)

    gtab = np.zeros((NG, 16), np.float32)
    gtab[:, 0:4] = cam.astype(np.float32)
    gtab[:, 4:10] = cnt
    gtab[:, 10] = va
    gtab[:, 11] = rec1

    lab16 = labels.astype(np.uint16)
    lab_bc = np.ascontiguousarray(np.broadcast_to(lab16[None, :], (128, NG)))

    e4 = np.zeros((4, 128, 128), np.float32)
    for j in range(4):
        for i in range(128):
            e4[j, i, 32 * j + i // 4] = 1.0
    e4b = e4.astype(ml_dtypes.bfloat16)

    ones1 = np.ones((128, 1), np.float32)
    ident = np.eye(128, dtype=ml_dtypes.bfloat16)

    in_maps = []
    for k in range(NCORES):
        g0 = k * G
        camk = cam[g0:g0 + G].reshape(-1)
        wslot = np.zeros((RT, 128, 128), np.float32)
        for t in range(RT):
            for i in range(128):
                row = 128 * t + i
                j0 = row - row % 4
                s = int(np.nonzero(camk[j0:j0 + 4] == camk[row])[0][0])
                wslot[t, i, 4 * (i // 4) + s] = 1.0
        in_maps.append({
            "x_sh": x[k * R:(k + 1) * R],
            "lab_bc": lab_bc,
            "lab_loc": np.ascontiguousarray(
                labels[g0:g0 + G].reshape(G, 1).astype(np.float32)),
            "gtab": gtab,
            "atab": np.ascontiguousarray(gtab[g0:g0 + G]),
            "e4b": e4b,
            "e4f": e4,
            "wslot": wslot.astype(ml_dtypes.bfloat16),
            "ones1": ones1,
            "ident": ident,
        })
    return in_maps


def kernel(input, target, camera_id):
    if "nc" not in _CACHE:
        _CACHE["nc"] = _build()
    nc = _CACHE["nc"]
    in_maps = _host_prep(input, target, camera_id)
    res = run_bass_kernel_spmd(nc, in_maps, core_ids=list(range(NCORES)))
    total = np.float64(0.0)
    for r in range(NCORES):
        total += np.float64(res.results[r]["loss_part"][0, 0])
    return np.float32(total)

